# revision 6
# baseline (speedup 1.0000x reference)
"""Trainium2 Bass kernel for nn_AppPreUserPGtrDocAttn (sparse_attention).

Strategy: pure data-parallel over the window dim N across 8 NeuronCores.
Each core computes 512 output windows (last core: 509 real + 3 discarded).
All weights are replicated; inputs are sharded/padded/transposed on host.
Compute dtype: bf16 matmuls with fp32 PSUM accumulation.

Per-core pipeline (feature-major / transposed activations throughout):
  A: xT[0:256, :]  = emb_app_w.T @ app_shard.T      (K=10000 streamed)
     xT[256:320,:] = emb_tim_w.T @ onehot(tim)      (one-hot gather matmul)
  B: s = attn_W.T @ xT                              ([1, 515] row vector)
  C: H[f] = tanh(s[f:f+512] + b[f]); w[f] = H[f]/sum_f|H[f]|
  D: yT = attn_fc_w @ xT;  outT = sum_f bcast(w[f]) * yT[:, f:f+512]
  E: out2T[0:256] = (outT + fc_b) * uid_emb; out2T[256:320] = ptim one-hot
     out2T[320] = 1  (bias row; dec_b is appended to dec_w.T on host)
  F: score = sigmoid(dec_w_aug.T.T @ out2T) streamed over 10240 cols
"""

import numpy as np

try:
    import concourse.bass as bass
except ImportError:  # pragma: no cover
    import sys

    sys.path.insert(0, "/opt/trn_rl_repo")
    import concourse.bass as bass

import ml_dtypes

import concourse.mybir as mybir
from concourse import bacc, bass_utils
from concourse.tile import TileContext

BF = ml_dtypes.bfloat16
F32 = mybir.dt.float32
BF16 = mybir.dt.bfloat16
AF = mybir.ActivationFunctionType
ALU = mybir.AluOpType

S = 4096            # sequence length
NWIN = S - 3        # 4093 windows
NCORES = 8
R = 512             # windows per core (last core: 509 real)
RH = R + 3          # x rows needed per core (halo)
RP = 520            # padded col count for xT/appT (512 + 8)
KAPP = 10000        # app vocab / contraction dim
KAPPP = 10240       # padded to 80 k-tiles of 128
NKT = KAPPP // 128  # 80
KB = 8              # k-tiles per DMA batch
NKB = NKT // KB     # 10
E = 256             # app emb dim
TE = 64             # tim emb dim
D = 320             # INPUT_SIZE
DP = 384            # padded feature dim (3 k-tiles of 128)
NOUT = 10000        # decoder outputs
NOUTP = 10240       # padded to 20 chunks of 512
GW = 2048           # out cols per group (4 chunks of 512)
NG = NOUTP // GW    # 5

_CACHE: dict = {}


def _build():
    nc = bacc.Bacc()

    appT_d = nc.declare_dram_parameter("appT", [KAPPP, RP], BF16, isOutput=False)
    wapp_d = nc.declare_dram_parameter("wapp", [KAPPP, E], BF16, isOutput=False)
    decw_d = nc.declare_dram_parameter("decw", [DP, NOUTP], BF16, isOutput=False)
    fcw_d = nc.declare_dram_parameter("fcw", [DP, E], BF16, isOutput=False)
    embt_d = nc.declare_dram_parameter("embt", [48, TE], BF16, isOutput=False)
    attnw_d = nc.declare_dram_parameter("attnw", [DP], BF16, isOutput=False)
    attnb_d = nc.declare_dram_parameter("attnb", [4], F32, isOutput=False)
    timv_d = nc.declare_dram_parameter("timv", [RP], BF16, isOutput=False)
    ptimv_d = nc.declare_dram_parameter("ptimv", [R], BF16, isOutput=False)
    iota_d = nc.declare_dram_parameter("iota", [128], F32, isOutput=False)
    uide_d = nc.declare_dram_parameter("uide", [E], F32, isOutput=False)
    fcb_d = nc.declare_dram_parameter("fcb", [E], F32, isOutput=False)
    out_d = nc.declare_dram_parameter("out", [R, NOUTP], BF16, isOutput=True)

    with TileContext(nc) as tc:
        with (
            tc.tile_pool(name="const", bufs=1) as const,
            tc.tile_pool(name="sb", bufs=1) as sb,
            tc.tile_pool(name="apool", bufs=3) as apool,
            tc.tile_pool(name="wpool", bufs=3) as wpool,
            tc.tile_pool(name="dpool", bufs=5) as dpool,
            tc.tile_pool(name="opool", bufs=3) as opool,
            tc.tile_pool(name="tmp", bufs=2) as tmp,
        ):
            # ---- constants / small inputs ----
            ones_sb = const.tile([1, 128], BF16)
            nc.vector.memset(ones_sb[:], 1.0)
            iota_sb = const.tile([128, 1], F32)
            nc.sync.dma_start(iota_sb[:], iota_d.rearrange("(p o) -> p o", o=1))
            uide_sb = const.tile([128, 2], F32)
            nc.sync.dma_start(uide_sb[:], uide_d.rearrange("(m p) -> p m", p=128))
            fcb_sb = const.tile([128, 2], F32)
            nc.sync.dma_start(fcb_sb[:], fcb_d.rearrange("(m p) -> p m", p=128))
            attnw_sb = const.tile([128, 3], BF16)
            nc.sync.dma_start(attnw_sb[:], attnw_d.rearrange("(t p) -> p t", p=128))
            attnb_sb = const.tile([1, 4], F32)
            nc.sync.dma_start(attnb_sb[:], attnb_d.rearrange("(o c) -> o c", o=1))
            embt_sb = const.tile([48, TE], BF16)
            nc.sync.dma_start(embt_sb[:], embt_d[:, :])
            timv_sb = const.tile([1, RP], BF16)
            nc.sync.dma_start(timv_sb[:], timv_d.rearrange("(o c) -> o c", o=1))
            ptimv_sb = const.tile([1, R], BF16)
            nc.sync.dma_start(ptimv_sb[:], ptimv_d.rearrange("(o c) -> o c", o=1))
            fcw_sb = const.tile([128, 3, E], BF16)
            nc.sync.dma_start(fcw_sb[:], fcw_d.rearrange("(t p) e -> p t e", p=128))

            # persistent activations
            xTa = sb.tile([128, 2, RP], BF16)      # x.T features 0:256
            xTt = sb.tile([TE, RP], BF16)          # x.T features 256:320
            s_sb = sb.tile([1, RP], F32)           # attention logits
            Hs = sb.tile([1, 4 * R], F32)          # tanh windows
            wf_sb = sb.tile([1, 4 * R], BF16)      # normalized weights
            yT = sb.tile([128, 2, RP], F32)        # fc_w @ x.T
            o2a = sb.tile([128, 2, R], BF16)       # out2.T rows 0:256
            o2t = sb.tile([128, R], BF16)          # out2.T rows 256:384

            # ---- stage A: xT[0:256] = wapp.T @ appT, streamed over K ----
            with tc.tile_pool(name="psA", bufs=1, space="PSUM") as psA:
                pxa0 = psA.tile([128, 512], F32)
                pxa1 = psA.tile([128, 512], F32)
                px80 = psA.tile([128, 8], F32)
                px81 = psA.tile([128, 8], F32)
                pxa = [pxa0, pxa1]
                px8 = [px80, px81]
                appT_r = appT_d.rearrange("(b k p) c -> b p k c", k=KB, p=128)
                wapp_r = wapp_d.rearrange("(b k p) e -> b p k e", k=KB, p=128)
                for b in range(NKB):
                    at = apool.tile([128, KB, RP], BF16, name="at")
                    wt = wpool.tile([128, KB, E], BF16, name="wt")
                    nc.gpsimd.dma_start(at[:], appT_r[b])
                    nc.gpsimd.dma_start(wt[:], wapp_r[b])
                    for k in range(KB):
                        kt = b * KB + k
                        start = kt == 0
                        stop = kt == NKT - 1
                        for mt in range(2):
                            lhsT = wt[:, k, mt * 128:(mt + 1) * 128]
                            nc.tensor.matmul(pxa[mt][:], lhsT, at[:, k, 0:512],
                                             start=start, stop=stop)
                            nc.tensor.matmul(px8[mt][:], lhsT, at[:, k, 512:RP],
                                             start=start, stop=stop)
                for mt in range(2):
                    nc.vector.tensor_copy(xTa[:, mt, 0:512], pxa[mt][:])
                    nc.vector.tensor_copy(xTa[:, mt, 512:RP], px8[mt][:])

            # ---- tim / ptim one-hot embedding gathers ----
            with tc.tile_pool(name="psT", bufs=1, space="PSUM") as psT:
                pb = psT.tile([48, 512], F32)
                pb8 = psT.tile([48, 8], F32)
                pt = psT.tile([TE, 512], F32)
                pt8 = psT.tile([TE, 8], F32)
                ppb = psT.tile([48, 512], F32)
                ppt = psT.tile([TE, 512], F32)
                oh = tmp.tile([48, RP], BF16, name="oh")
                ohp = tmp.tile([48, R], BF16, name="ohp")

                nc.tensor.matmul(pb[:], ones_sb[0:1, 0:48], timv_sb[0:1, 0:512],
                                 start=True, stop=True)
                nc.tensor.matmul(pb8[:], ones_sb[0:1, 0:48], timv_sb[0:1, 512:RP],
                                 start=True, stop=True)
                nc.vector.tensor_scalar(oh[:, 0:512], pb[:], iota_sb[0:48, :],
                                        None, op0=ALU.is_equal)
                nc.vector.tensor_scalar(oh[:, 512:RP], pb8[:], iota_sb[0:48, :],
                                        None, op0=ALU.is_equal)
                nc.tensor.matmul(pt[:], embt_sb[:], oh[:, 0:512],
                                 start=True, stop=True)
                nc.tensor.matmul(pt8[:], embt_sb[:], oh[:, 512:RP],
                                 start=True, stop=True)
                nc.vector.tensor_copy(xTt[:, 0:512], pt[:])
                nc.vector.tensor_copy(xTt[:, 512:RP], pt8[:])

                nc.tensor.matmul(ppb[:], ones_sb[0:1, 0:48], ptimv_sb[:],
                                 start=True, stop=True)
                nc.vector.tensor_scalar(ohp[:], ppb[:], iota_sb[0:48, :],
                                        None, op0=ALU.is_equal)
                nc.tensor.matmul(ppt[:], embt_sb[:], ohp[:],
                                 start=True, stop=True)
                nc.vector.tensor_copy(o2t[0:TE, :], ppt[:])
                nc.vector.memset(o2t[TE:128, :], 0.0)
                nc.vector.memset(o2t[TE:TE + 1, :], 1.0)   # bias row (dec_b)

            # ---- stage B: s = attn_W.T @ xT ; yT = fc_w @ xT ----
            with tc.tile_pool(name="psB", bufs=1, space="PSUM") as psB:
                ps = psB.tile([1, 512], F32)
                ps8 = psB.tile([1, 8], F32)
                py0 = psB.tile([128, 512], F32)
                py08 = psB.tile([128, 8], F32)
                py1 = psB.tile([128, 512], F32)
                py18 = psB.tile([128, 8], F32)
                py = [py0, py1]
                py8 = [py08, py18]
                xts = [xTa[:, 0, :], xTa[:, 1, :], xTt[:, :]]
                klens = [128, 128, TE]
                for kt in range(3):
                    xt, kl = xts[kt], klens[kt]
                    st, sp = kt == 0, kt == 2
                    nc.tensor.matmul(ps[:], attnw_sb[0:kl, kt:kt + 1],
                                     xt[0:kl, 0:512], start=st, stop=sp)
                    nc.tensor.matmul(ps8[:], attnw_sb[0:kl, kt:kt + 1],
                                     xt[0:kl, 512:RP], start=st, stop=sp)
                    for mt in range(2):
                        lhsT = fcw_sb[0:kl, kt, mt * 128:(mt + 1) * 128]
                        nc.tensor.matmul(py[mt][:], lhsT, xt[0:kl, 0:512],
                                         start=st, stop=sp)
                        nc.tensor.matmul(py8[mt][:], lhsT, xt[0:kl, 512:RP],
                                         start=st, stop=sp)
                nc.vector.tensor_copy(s_sb[:, 0:512], ps[:])
                nc.vector.tensor_copy(s_sb[:, 512:RP], ps8[:])
                for mt in range(2):
                    nc.vector.tensor_copy(yT[:, mt, 0:512], py[mt][:])
                    nc.vector.tensor_copy(yT[:, mt, 512:RP], py8[mt][:])

            # ---- stage C: window weights ----
            for f in range(4):
                nc.scalar.activation(Hs[0:1, f * R:(f + 1) * R],
                                     s_sb[0:1, f:f + R], AF.Tanh,
                                     bias=attnb_sb[0:1, f:f + 1])
            absH = tmp.tile([1, 4 * R], F32, name="absH")
            nc.scalar.activation(absH[:], Hs[:], AF.Abs)
            l1a = tmp.tile([1, R], F32, name="l1a")
            l1b = tmp.tile([1, R], F32, name="l1b")
            rec = tmp.tile([1, R], F32, name="rec")
            nc.vector.tensor_add(l1a[:], absH[0:1, 0:R], absH[0:1, R:2 * R])
            nc.vector.tensor_add(l1b[:], absH[0:1, 2 * R:3 * R], absH[0:1, 3 * R:4 * R])
            nc.vector.tensor_add(l1a[:], l1a[:], l1b[:])
            nc.vector.reciprocal(rec[:], l1a[:])
            for f in range(4):
                nc.vector.tensor_mul(wf_sb[0:1, f * R:(f + 1) * R],
                                     Hs[0:1, f * R:(f + 1) * R], rec[:])

            # ---- stage D/E: outT = sum_f bcast(w[f]) * yT shifted; scale ----
            with tc.tile_pool(name="psW", bufs=1, space="PSUM") as psW:
                pw = [psW.tile([128, R], F32, name=f"pw{f}") for f in range(4)]
                for f in range(4):
                    nc.tensor.matmul(pw[f][:], ones_sb[0:1, 0:128],
                                     wf_sb[0:1, f * R:(f + 1) * R],
                                     start=True, stop=True)
                for mt in range(2):
                    acc = tmp.tile([128, R], F32, name="acc")
                    prod = tmp.tile([128, R], F32, name="prod")
                    nc.vector.tensor_mul(acc[:], pw[0][:], yT[:, mt, 0:R])
                    for f in range(1, 4):
                        nc.vector.tensor_mul(prod[:], pw[f][:], yT[:, mt, f:f + R])
                        nc.vector.tensor_add(acc[:], acc[:], prod[:])
                    nc.vector.tensor_scalar(o2a[:, mt, :], acc[:],
                                            fcb_sb[:, mt:mt + 1],
                                            uide_sb[:, mt:mt + 1],
                                            op0=ALU.add, op1=ALU.mult)

            # ---- stage F: score = sigmoid(out2T.T @ decw) ----
            with tc.tile_pool(name="psF", bufs=4, space="PSUM") as psF:
                o2 = [o2a[:, 0, :], o2a[:, 1, :], o2t[:, :]]
                decw_r = decw_d.rearrange("(t p) (g c) -> g p t c", p=128, c=GW)
                for g in range(NG):
                    dw = dpool.tile([128, 3, GW], BF16, name="dw")
                    nc.gpsimd.dma_start(dw[:], decw_r[g])
                    for mt in range(4):
                        ob = opool.tile([128, GW], BF16, name="ob")
                        for sub in range(4):
                            pf = psF.tile([128, 512], F32, name="pf")
                            for kt in range(3):
                                nc.tensor.matmul(
                                    pf[:],
                                    o2[kt][:, mt * 128:(mt + 1) * 128],
                                    dw[:, kt, sub * 512:(sub + 1) * 512],
                                    start=(kt == 0), stop=(kt == 2))
                            nc.scalar.activation(ob[:, sub * 512:(sub + 1) * 512],
                                                 pf[:], AF.Sigmoid)
                        nc.gpsimd.dma_start(
                            out_d[mt * 128:(mt + 1) * 128, g * GW:(g + 1) * GW],
                            ob[:])

    nc.finalize()
    return nc


def _host_prep(tim, app, uid, ptim, emb_tim_w, emb_uid_w, emb_app_w,
               attn_W, attn_b, attn_fc_w, attn_fc_b, dec_w, dec_b):
    """Shard + pad + transpose + cast all inputs; returns in_maps for 8 cores."""
    app = np.asarray(app, dtype=np.float32)
    tim = np.asarray(tim).reshape(-1)
    ptim = np.asarray(ptim).reshape(-1)
    uid = int(np.asarray(uid).reshape(-1)[0])

    app_bf = app.astype(BF)

    wapp = np.zeros((KAPPP, E), dtype=BF)
    wapp[:KAPP] = np.asarray(emb_app_w, dtype=np.float32).astype(BF)

    decw = np.zeros((DP, NOUTP), dtype=BF)
    dwT = np.ascontiguousarray(np.asarray(dec_w, dtype=np.float32).T)  # [320, 10000]
    decw[:D, :NOUT] = dwT.astype(BF)
    decw[D, :NOUT] = np.asarray(dec_b, dtype=np.float32).astype(BF)

    fcw = np.zeros((DP, E), dtype=BF)
    fcw[:D] = np.ascontiguousarray(
        np.asarray(attn_fc_w, dtype=np.float32).T).astype(BF)

    embt = np.asarray(emb_tim_w, dtype=np.float32).astype(BF)

    attnw = np.zeros((DP,), dtype=BF)
    attnw[:D] = np.asarray(attn_W, dtype=np.float32).reshape(-1).astype(BF)

    attnb = np.asarray(attn_b, dtype=np.float32).reshape(-1).copy()

    iota = np.arange(128, dtype=np.float32)
    uide = np.asarray(emb_uid_w, dtype=np.float32)[uid].copy()
    fcb = np.asarray(attn_fc_b, dtype=np.float32).reshape(-1).copy()

    in_maps = []
    for c in range(NCORES):
        r0 = c * R
        r1 = min(r0 + RH, S)
        n = r1 - r0
        appT = np.zeros((KAPPP, RP), dtype=BF)
        appT[:KAPP, :n] = app_bf[r0:r1].T

        timv = np.zeros((RP,), dtype=BF)
        timv[:n] = tim[r0:r1].astype(BF)

        ptimv = np.zeros((R,), dtype=BF)
        np_ = min(r0 + R, NWIN) - r0
        ptimv[:np_] = ptim[r0:r0 + np_].astype(BF)

        in_maps.append({
            "appT": appT, "wapp": wapp, "decw": decw, "fcw": fcw,
            "embt": embt, "attnw": attnw, "attnb": attnb, "timv": timv,
            "ptimv": ptimv, "iota": iota, "uide": uide, "fcb": fcb,
        })
    return in_maps


def kernel(tim, app, loc, uid, ptim, emb_tim_w, emb_uid_w, emb_app_w,
           attn_W, attn_b, attn_fc_w, attn_fc_b, dec_w, dec_b,
           _trace=False, _trace_kwargs=None):
    if "nc" not in _CACHE:
        _CACHE["nc"] = _build()
    nc = _CACHE["nc"]

    in_maps = _host_prep(tim, app, uid, ptim, emb_tim_w, emb_uid_w, emb_app_w,
                         attn_W, attn_b, attn_fc_w, attn_fc_b, dec_w, dec_b)

    kw = {}
    if _trace:
        kw["trace"] = True
        if _trace_kwargs:
            kw.update(_trace_kwargs)
    res = bass_utils.run_bass_kernel_spmd(nc, in_maps, core_ids=list(range(NCORES)), **kw)
    _CACHE["last_result"] = res

    outs = []
    for c in range(NCORES):
        nrows = R if c < NCORES - 1 else NWIN - (NCORES - 1) * R
        outs.append(np.asarray(res.results[c]["out"])[:nrows, :NOUT])
    return np.concatenate(outs, axis=0).astype(np.float32)
